# revision 7
# baseline (speedup 1.0000x reference)
"""Trainium2 Bass kernel for a 2-layer LSTM extractor.

Reference computation (see problem):
  x: [512, 1, 512, 28] -> squeeze -> [B=512, T=512, D=28]
  layer0: LSTM(D=28 -> H=128), layer1: LSTM(128 -> 128)
  output: final hidden state of layer1, [512, 128]

Strategy:
  - Data parallel: batch 512 sharded 8 ways -> B=64 per NeuronCore.
  - Per core, both layers fused in one time loop, layer1 skewed one step
    behind layer0 so its work fills engine gaps.
  - Gate-transposed layout everywhere: states h/c stored [H=128 part, B=64
    free]; gate pre-activations computed as [4H-chunk part, B free] via
    matmuls with stationary weight chunks lhsT=[K,128] and moving rhs=h.
    No transposes anywhere on-chip.
  - All matmuls in bf16 (weights + h + x); PSUM accumulation and the
    cell state c stay fp32.  fp32 matmuls on TRN2 run at 1/4 rate and
    double the HW matmul count, which made the PE the bottleneck.
  - All layout work is done host-side: weights arrive pre-transposed,
    x arrives pre-transposed time-major ([33, T*B] with a ones row for
    the folded L0 bias), and the output is un-transposed on the host.
  - L0 biases folded into the x-projection via the ones-row (K=33).
    L1 biases applied via the ACT per-partition bias operand.
"""

import os
import sys

import numpy as np

for _p in ("/opt/trn_rl_repo", os.path.expanduser("~/.axon_site/_ro/trn_rl_repo")):
    if os.path.isdir(_p) and _p not in sys.path:
        sys.path.insert(0, _p)

import concourse.bacc as bacc
import concourse.tile as tile
from concourse import mybir
from concourse.bass_utils import run_bass_kernel_spmd

B_FULL, T_FULL, D, H = 512, 512, 28, 128
NCORES = 8
B = B_FULL // NCORES  # 64 per core
G4 = 4 * H  # 512
P = 128
F32 = mybir.dt.float32
BF16 = mybir.dt.bfloat16
AF = mybir.ActivationFunctionType

# weight chunk g (PyTorch gate order i,f,g,o) -> psum column block.
# Sigmoid gates (i,f,o) are kept contiguous so one ACT op covers them.
COL_OF = [0, 1, 3, 2]  # i->0, f->1, g->3, o->2
KA = 33  # augmented contraction dim for the L0 x-projection (28 x + pad + bias)


def _emit(nc, tc, t_steps):
    T = t_steps
    xT = nc.dram_tensor("xT", [KA, T * B], BF16, kind="ExternalInput").ap()
    wih0T = nc.dram_tensor("wih0T", [KA, G4], BF16, kind="ExternalInput").ap()
    whh0T = nc.dram_tensor("whh0T", [P, G4], BF16, kind="ExternalInput").ap()
    wih1T = nc.dram_tensor("wih1T", [P, G4], BF16, kind="ExternalInput").ap()
    whh1T = nc.dram_tensor("whh1T", [P, G4], BF16, kind="ExternalInput").ap()
    b1aug = nc.dram_tensor("b1aug", [KA, G4], BF16, kind="ExternalInput").ap()
    outT = nc.dram_tensor("outT", [H, B], F32, kind="ExternalOutput").ap()

    from contextlib import ExitStack

    es = ExitStack()
    with es:
        consts = es.enter_context(tc.tile_pool(name="consts", bufs=1))
        ps0p = es.enter_context(tc.tile_pool(name="ps0p", bufs=2, space="PSUM"))
        ps1p = es.enter_context(tc.tile_pool(name="ps1p", bufs=2, space="PSUM"))
        states = es.enter_context(tc.tile_pool(name="states", bufs=3))
        work = es.enter_context(tc.tile_pool(name="work", bufs=3))

        xTs = consts.tile([KA, T * B], BF16)
        wih0Ts = consts.tile([KA, G4], BF16)
        whh0Ts = consts.tile([P, G4], BF16)
        wih1Ts = consts.tile([P, G4], BF16)
        whh1Ts = consts.tile([P, G4], BF16)
        b1augs = consts.tile([KA, G4], BF16)
        nc.sync.dma_start(out=xTs[:], in_=xT)
        nc.sync.dma_start(out=wih0Ts[:], in_=wih0T)
        nc.sync.dma_start(out=whh0Ts[:], in_=whh0T)
        nc.sync.dma_start(out=wih1Ts[:], in_=wih1T)
        nc.sync.dma_start(out=whh1Ts[:], in_=whh1T)
        nc.sync.dma_start(out=b1augs[:], in_=b1aug)

        # ---- main time loop; L0 at t=k, L1 at t=k-1 ----
        h0 = states.tile([P, B], BF16, tag="h0")
        c0 = states.tile([P, B], F32, tag="c0")
        h1 = states.tile([P, B], BF16, tag="h1")
        c1 = states.tile([P, B], F32, tag="c1")
        for t_ in (h0, c0, h1, c1):
            nc.vector.memset(t_[:], 0.0)

        for k in range(t_steps + 1):
            h0_prev, h1_prev = h0, h1
            if k < t_steps:
                # The L0 recurrence (h0 -> gates -> c0 -> h0) is the critical
                # cycle; pin it ahead of L1's ops in each engine queue so the
                # scheduler never wedges an off-chain op between its stages.
                with tc.high_priority(offset=40):
                    rhs_x = xTs[:, k * B : (k + 1) * B]
                    ps0 = ps0p.tile([P, 4 * B], F32, tag="ps0")
                    # one accumulation group per psum bank: start only on the
                    # first matmul, stop on the last
                    for g in range(4):
                        cb = COL_OF[g] * B
                        nc.tensor.matmul(
                            ps0[:, cb : cb + B],
                            lhsT=wih0Ts[:, g * P : (g + 1) * P],
                            rhs=rhs_x,
                            start=(g == 0),
                            stop=False,
                        )
                    for g in range(4):
                        cb = COL_OF[g] * B
                        nc.tensor.matmul(
                            ps0[:, cb : cb + B],
                            lhsT=whh0Ts[:, g * P : (g + 1) * P],
                            rhs=h0_prev[:],
                            start=False,
                            stop=(g == 3),
                        )
                    sifo = work.tile([P, 3 * B], F32, tag="sifo")
                    nc.scalar.activation(sifo[:], ps0[:, 0 : 3 * B], AF.Sigmoid)
                    tg = work.tile([P, B], F32, tag="tg")
                    nc.scalar.activation(tg[:], ps0[:, 3 * B : 4 * B], AF.Tanh)
                    fc = work.tile([P, B], F32, tag="fc")
                    nc.vector.tensor_mul(fc[:], sifo[:, B : 2 * B], c0[:])
                    ig = work.tile([P, B], F32, tag="ig")
                    nc.vector.tensor_mul(ig[:], sifo[:, 0:B], tg[:])
                    c0 = states.tile([P, B], F32, tag="c0")
                    nc.vector.tensor_add(c0[:], fc[:], ig[:])
                    tc0 = work.tile([P, B], F32, tag="tc0")
                    nc.scalar.activation(tc0[:], c0[:], AF.Tanh)
                    h0 = states.tile([P, B], BF16, tag="h0")
                    nc.vector.tensor_mul(h0[:], sifo[:, 2 * B : 3 * B], tc0[:])

            if k >= 1:
                # L1 biases enter via the xT ones-row: b1aug rows 0:32 are
                # zero, row 32 is b_ih1+b_hh1, and any rhs_x column block has
                # ones in row 32.
                rhs_b = xTs[:, (k - 1) * B : k * B]
                ps1 = ps1p.tile([P, 4 * B], F32, tag="ps1")
                for g in range(4):
                    cb = COL_OF[g] * B
                    nc.tensor.matmul(
                        ps1[:, cb : cb + B],
                        lhsT=b1augs[:, g * P : (g + 1) * P],
                        rhs=rhs_b,
                        start=(g == 0),
                        stop=False,
                    )
                for g in range(4):
                    cb = COL_OF[g] * B
                    nc.tensor.matmul(
                        ps1[:, cb : cb + B],
                        lhsT=wih1Ts[:, g * P : (g + 1) * P],
                        rhs=h0_prev[:],
                        start=False,
                        stop=False,
                    )
                for g in range(4):
                    cb = COL_OF[g] * B
                    nc.tensor.matmul(
                        ps1[:, cb : cb + B],
                        lhsT=whh1Ts[:, g * P : (g + 1) * P],
                        rhs=h1_prev[:],
                        start=False,
                        stop=(g == 3),
                    )
                sifo1 = work.tile([P, 3 * B], F32, tag="sifo1")
                nc.scalar.activation(sifo1[:], ps1[:, 0 : 3 * B], AF.Sigmoid)
                tg1 = work.tile([P, B], F32, tag="tg1")
                nc.scalar.activation(tg1[:], ps1[:, 3 * B : 4 * B], AF.Tanh)
                fc1 = work.tile([P, B], F32, tag="fc1")
                nc.vector.tensor_mul(fc1[:], sifo1[:, B : 2 * B], c1[:])
                ig1 = work.tile([P, B], F32, tag="ig1")
                nc.vector.tensor_mul(ig1[:], sifo1[:, 0:B], tg1[:])
                c1 = states.tile([P, B], F32, tag="c1")
                nc.vector.tensor_add(c1[:], fc1[:], ig1[:])
                tc1 = work.tile([P, B], F32, tag="tc1")
                nc.scalar.activation(tc1[:], c1[:], AF.Tanh)
                h1 = states.tile([P, B], BF16, tag="h1")
                nc.vector.tensor_mul(h1[:], sifo1[:, 2 * B : 3 * B], tc1[:])

        # ---- output: [H,B] fp32, un-transposed host-side ----
        h1f = work.tile([P, B], F32, tag="h1f")
        nc.scalar.copy(out=h1f[:], in_=h1[:])
        nc.sync.dma_start(out=outT, in_=h1f[:])


_NC_CACHE = {}


def build_nc(t_steps=T_FULL):
    if t_steps in _NC_CACHE:
        return _NC_CACHE[t_steps]
    nc = bacc.Bacc(
        "TRN2",
        target_bir_lowering=False,
        debug=False,
        enable_asserts=False,
        num_devices=NCORES,
    )
    with tile.TileContext(nc) as tc:
        _emit(nc, tc, t_steps)
    nc.compile()
    _NC_CACHE[t_steps] = nc
    return nc


def make_in_maps(inputs, t_steps=T_FULL):
    from ml_dtypes import bfloat16

    x = np.asarray(inputs["x"], dtype=np.float32).reshape(B_FULL, T_FULL, D)
    x = x[:, :t_steps, :]
    wih0T = np.zeros((KA, G4), np.float32)
    wih0T[0:D] = np.asarray(inputs["W_ih0"], np.float32).T
    wih0T[KA - 1] = np.asarray(inputs["b_ih0"], np.float32) + np.asarray(
        inputs["b_hh0"], np.float32
    )
    b1 = np.asarray(inputs["b_ih1"], np.float32) + np.asarray(
        inputs["b_hh1"], np.float32
    )
    b1aug = np.zeros((KA, G4), np.float32)
    b1aug[KA - 1] = b1
    shared = {
        "wih0T": wih0T.astype(bfloat16),
        "whh0T": np.ascontiguousarray(
            np.asarray(inputs["W_hh0"], np.float32).T
        ).astype(bfloat16),
        "wih1T": np.ascontiguousarray(
            np.asarray(inputs["W_ih1"], np.float32).T
        ).astype(bfloat16),
        "whh1T": np.ascontiguousarray(
            np.asarray(inputs["W_hh1"], np.float32).T
        ).astype(bfloat16),
        "b1aug": b1aug.astype(bfloat16),
    }
    in_maps = []
    for c in range(NCORES):
        m = dict(shared)
        xc = x[c * B : (c + 1) * B]  # [B, T, D]
        xt = np.zeros((KA, t_steps, B), np.float32)
        xt[0:D] = xc.transpose(2, 1, 0)
        xt[KA - 1] = 1.0
        m["xT"] = xt.reshape(KA, t_steps * B).astype(bfloat16)
        in_maps.append(m)
    return in_maps


def run(inputs, t_steps=T_FULL, trace=False, **kwargs):
    nc = build_nc(t_steps)
    in_maps = make_in_maps(inputs, t_steps)
    res = run_bass_kernel_spmd(
        nc, in_maps, core_ids=list(range(NCORES)), trace=trace, **kwargs
    )
    outs = [np.asarray(res.results[c]["outT"]).T for c in range(NCORES)]
    return np.concatenate(outs, axis=0).astype(np.float32), res


def kernel(**inputs):
    out, _ = run(inputs)
    return out
